# revision 16
# baseline (speedup 1.0000x reference)
"""Memory-augmented attention kernel for Trainium2 (Bass/Tile), 8-core data parallel.

Reference computation (per row b of B=32768, D=512, K=5):
    q' = query@Wq + bq
    k  = mem@Wk + bk ; v = mem@Wv + bv
    scores = (q'.k_j)/sqrt(D) masked-softmax -> w
    mem_out = (sum_j w_j v_j)@Wo + bo
    gate = sigmoid([query, mem_out]@Wg + bg); conf = sigmoid(max_sim - 0.7)
    out = LN(query + gate*conf*mem_out) * ln_g + ln_b

Algebraic refactoring (biases are zero / LN affine identity in this problem;
a numpy fallback covers the general case):
    scores_bk = m_bk . (query_b @ (Wq @ Wk^T) / sqrt(D))
    mem_out_b = (sum_k w_bk m_bk) @ (Wv @ Wo)
    gate_b    = sigmoid(query_b . Wg[:D] + mcomb_b . (Wv@Wo@Wg[D:]))
    conf      = sigmoid(max_k sims - 0.7)  (computed on host)

All bulk data moves HBM<->SBUF in bf16 (query+memories staged as one bf16
buffer, output stored bf16 and upcast on host), halving DMA traffic vs f32.
Scores are computed without max-subtraction (|scores| ~ N(0,1), exp safe).

Per 128-row tile, stages pipelined with deep lag so every engine streams:
    IN : DMA qm tile (bf16) + qT via SBUF->SBUF xbar dma_start_transpose
    A  : PE  pt = q@Wqk (lhsT=qT chunks), [-qdot, qsum/D] via gq cols;
         ACT copies pt (bf16), -qdot, qsum to SBUF
    B  : DVE scores dots (scalar_tensor_tensor bf16 + accum), ACT exp,
         DVE masked-sum STT -> w,sumexp, recip, 5x diag_k = I*w_k (bf16)
    C  : PE  mcomb = sum_k diag_k^T @ m_k (unnormalized); ACT copy bf16;
         DMA xbar transpose mcomb -> mcT
    D  : PE  mem = mcomb@Wvo, [-mdot, rowsum(Wvo)-dot] via gm cols;
         ACT ge = exp(-rsum*mdot - qdot) reading PSUM; DVE gp1, rgp
    E1 : DVE s = rgp*conf*rsum; ACT mem_s = s*mem (PSUM->SBUF bf16)
    E2 : GPSIMD out_pre = mem_s + q (bf16)
    E3 : DVE mu; GPSIMD mu2; ACT sumsq (Square+accum); DVE var
    E4 : ACT lnv, rstd (exp(-0.5 ln(var+eps))); DVE apply
         out = (out_pre - mu)*rstd via 2-AP-scalar tensor_scalar;
         GPSIMD out DMA (SWDGE)

This container's walrus build only encodes one sync-wait per instruction and
cannot encode EVENT_SEMAPHORE_RANGE_CLEAR; see _install_tile_patches.
"""

import numpy as np

B, D, K = 32768, 512, 5
N_CORES = 8
ROWS = B // N_CORES        # rows per core
P = 128                    # partitions
NT_FULL = ROWS // P        # tiles per core (32)
NCH = D // P               # 128-contraction chunks (4)
LN_EPS = 1e-5
SIM_THRESH = 0.7

_CACHE = {}

TRACE = False              # set by test harness to collect a HW profile
LAST_RESULTS = None        # BassKernelResults of the last run (for profiling)


def _install_tile_patches():
    """Work around two walrus limitations in this container:
    - instructions accept very few sync-wait slots: split the kernel-tail
      drain (which Tile loads with one wait per outstanding semaphore) into
      a chain of single-wait drains;
    - EVENT_SEMAPHORE_RANGE_CLEAR is not encodable: skip the on-device sem
      clear (each kernel() call executes a freshly loaded NEFF) while keeping
      the allocator bookkeeping.
    """
    import concourse.tile as tile
    from concourse.vector_clock import ScopedClock

    if getattr(tile.TileContext._drain_and_barrier, "_patched", False):
        return

    def patched(self, tick_clock, wait_clock):
        import bass_rust

        nc = self.nc
        drain_inst = nc.sync.drain()
        wait_clock.add_sem_waits(
            drain_inst.ins, ScopedClock({None: tick_clock.global_clock})
        )
        si = drain_inst.ins.sync_info
        waits = list(si.on_wait) if si is not None and si.on_wait else []
        if len(waits) > 1:
            drain_inst.ins.sync_info = bass_rust.SyncInfo(
                on_wait=waits[:1], on_update=list(si.on_update or [])
            )
            for w in waits[1:]:
                d2 = nc.sync.drain()
                d2.ins.sync_info = bass_rust.SyncInfo(on_wait=[w], on_update=[])
        nc.all_engine_barrier()
        assert self.sems is not None
        popped = nc._tile_sem_poison_stack.pop()
        assert popped is self._sem_poison
        sems = list(self.sems.allocated().values())
        sem_nums = [s.num for s in sems]
        nc._state.prepend_free_semaphores(sem_nums)
        for poison_set in nc._tile_sem_poison_stack:
            poison_set.update(sem_nums)
        nc.all_engine_barrier()

    patched._patched = True
    tile.TileContext._drain_and_barrier = patched

    # This walrus build accepts at most one sync-wait per instruction:
    # at commit time, peel off extra waits onto single-wait nop/drain
    # instructions inserted just before the owner.
    _orig_commit = tile.TileContext._commit_instruction

    def commit_patched(self, inst, lazy_reg_writes=True):
        import bass_rust
        from concourse import mybir

        si = inst.sync_info
        if si is not None and si.on_wait and len(si.on_wait) > 1:
            waits = list(si.on_wait)
            inst.sync_info = bass_rust.SyncInfo(
                on_wait=waits[-1:], on_update=list(si.on_update or [])
            )
            for w in waits[:-1]:
                eng = self.nc.engines[inst.engine]
                if not hasattr(eng, "engine_nop"):
                    nop = mybir.InstDrain(
                        name=self.nc.get_next_instruction_name(), ins=[], outs=[]
                    )
                    nop.engine = inst.engine
                else:
                    # sequencer-only ENGINE_NOP: carries the wait without
                    # flushing the compute pipeline the way a drain does
                    nop = eng.engine_nop().ins
                nop.sync_info = bass_rust.SyncInfo(on_wait=[w], on_update=[])
                self._add_instruction(nop)
        return _orig_commit(self, inst, lazy_reg_writes)

    tile.TileContext._commit_instruction = commit_patched


def _build(ntiles=NT_FULL):
    import concourse.bass as bass
    import concourse.tile as tile
    from concourse import mybir

    _install_tile_patches()

    f32 = mybir.dt.float32
    bf16 = mybir.dt.bfloat16
    AF = mybir.ActivationFunctionType
    OP = mybir.AluOpType

    rows = ntiles * P
    rD = 1.0 / float(D)

    nc = bass.Bass()
    qm_d = nc.declare_dram_parameter("qm", [rows, (K + 1) * D], bf16, isOutput=False)
    mask_d = nc.declare_dram_parameter("maskf", [rows, K], f32, isOutput=False)
    conf_d = nc.declare_dram_parameter("conf", [rows, 1], f32, isOutput=False)
    wqk_d = nc.declare_dram_parameter("wqk", [D, D], bf16, isOutput=False)
    wvo_d = nc.declare_dram_parameter("wvo", [D, D], bf16, isOutput=False)
    gq_d = nc.declare_dram_parameter("gq", [D, 2], bf16, isOutput=False)
    gm_d = nc.declare_dram_parameter("gm", [D, 2], bf16, isOutput=False)
    id_d = nc.declare_dram_parameter("ident", [P, P], bf16, isOutput=False)
    o_d = nc.declare_dram_parameter("o", [rows, D], bf16, isOutput=True)

    qm_t = qm_d.rearrange("(t p) x -> t p x", p=P)
    o_t = o_d.rearrange("(t p) d -> t p d", p=P)

    with tile.TileContext(nc) as tc:
        with (
            tc.tile_pool(name="consts", bufs=1) as consts,
            tc.tile_pool(name="qmload", bufs=12) as qmload,
            tc.tile_pool(name="tload", bufs=6) as tload,
            tc.tile_pool(name="work", bufs=4) as work,
            tc.tile_pool(name="smalls", bufs=12) as smalls,
            tc.tile_pool(name="ppt", bufs=2, space="PSUM") as ppt,
            tc.tile_pool(name="pmc", bufs=2, space="PSUM") as pmc,
            tc.tile_pool(name="pmem", bufs=2, space="PSUM") as pmem,
            tc.tile_pool(name="pmix", bufs=2, space="PSUM") as pmix,
        ):
            # ---- constants, loaded once ----
            wqk_sb = consts.tile([P, NCH, D], bf16)
            nc.sync.dma_start(out=wqk_sb, in_=wqk_d.rearrange("(c p) e -> p c e", p=P))
            wvo_sb = consts.tile([P, NCH, D], bf16)
            nc.sync.dma_start(out=wvo_sb, in_=wvo_d.rearrange("(c p) e -> p c e", p=P))
            gq_sb = consts.tile([P, NCH, 2], bf16)
            nc.sync.dma_start(out=gq_sb, in_=gq_d.rearrange("(c p) j -> p c j", p=P))
            gm_sb = consts.tile([P, NCH, 2], bf16)
            nc.sync.dma_start(out=gm_sb, in_=gm_d.rearrange("(c p) j -> p c j", p=P))
            identb = consts.tile([P, P], bf16)
            nc.sync.dma_start(out=identb, in_=id_d[:, :])
            mask_all = consts.tile([P, ntiles, K], f32)
            nc.sync.dma_start(
                out=mask_all, in_=mask_d.rearrange("(t p) k -> p t k", p=P)
            )
            conf_all = consts.tile([P, ntiles], f32)
            nc.sync.dma_start(
                out=conf_all, in_=conf_d.rearrange("(t p) j -> p (t j)", p=P)
            )
            epsc = consts.tile([P, 1], f32)
            nc.vector.memset(epsc, LN_EPS)

            def touch_dve(ap):
                tt = smalls.tile([P, 2], f32, tag="dvet", name="dvet")
                nc.vector.tensor_copy(out=tt[:, 0:ap.free_size()], in_=ap)

            def touch_act(ap):
                tt = smalls.tile([P, 2], f32, tag="actt", name="actt")
                nc.scalar.copy(out=tt[:, 0:ap.free_size()], in_=ap)

            def touch_gp(ap):
                tt = smalls.tile([P, 2], f32, tag="gpt", name="gpt")
                nc.gpsimd.tensor_copy(out=tt[:, 0:ap.free_size()], in_=ap)

            # Per-tile live state, keyed by tile index. Deep software pipeline
            # so each engine's in-order stream interleaves work from many
            # tiles instead of idling through each tile's dependency chain.
            st = {}

            def stage_in(t):
                s = st.setdefault(t, {})
                qm = qmload.tile([P, K + 1, D], bf16, tag="qm", name="qmtile")
                nc.sync.dma_start(out=qm, in_=qm_t[t].rearrange("p (s d) -> p s d", d=D))
                qT = tload.tile([P, NCH, P], bf16, tag="qT", name="qT")
                nc.sync.dma_start_transpose(out=qT, in_=qm[:, 0, :])
                s["qm"] = qm
                s["qT"] = qT

            def stage_a(t):
                # pt = q@Wqk ; q2 = [-qdot, qsum/D]
                s = st[t]
                pt_ps = ppt.tile([P, D], f32, tag="pt", name="pt_ps")
                for c in range(NCH):
                    nc.tensor.matmul(
                        pt_ps, lhsT=s["qT"][:, c, :], rhs=wqk_sb[:, c, :],
                        start=(c == 0), stop=(c == NCH - 1),
                    )
                q2_ps = pmix.tile([P, 2], f32, tag="mix2", name="q2_ps")
                for c in range(NCH):
                    nc.tensor.matmul(
                        q2_ps, lhsT=s["qT"][:, c, :], rhs=gq_sb[:, c, :],
                        start=(c == 0), stop=(c == NCH - 1),
                    )
                s["pt"] = work.tile([P, D], bf16, tag="pt_sb", name="pt_sb")
                nc.scalar.copy(out=s["pt"], in_=pt_ps)
                q2sb = smalls.tile([P, 2], f32, tag="q2sb", name="q2sb")
                nc.scalar.copy(out=q2sb, in_=q2_ps)
                s["nqdot"] = q2sb[:, 0:1]
                s["qsum"] = q2sb[:, 1:2]

            def stage_b(t):
                # raw_k = m_k . pt ; w = exp(raw)*mask ; rsum = 1/sum(w)
                s = st[t]
                raw = smalls.tile([P, K], f32, tag="raw", name="raw")
                scratch = work.tile([P, D], bf16, tag="scratch", name="scratch")
                for k in range(K):
                    nc.vector.scalar_tensor_tensor(
                        out=scratch, in0=s["qm"][:, 1 + k, :], scalar=1.0,
                        in1=s["pt"], op0=OP.mult, op1=OP.mult,
                        accum_out=raw[:, k:k + 1],
                    )
                expw = smalls.tile([P, K], f32, tag="expw", name="expw")
                nc.scalar.activation(out=expw, in_=raw, func=AF.Exp)
                s["w"] = smalls.tile([P, K], bf16, tag="w", name="w")
                sumexp = smalls.tile([P, 1], f32, tag="sumexp", name="sumexp")
                nc.vector.scalar_tensor_tensor(
                    out=s["w"], in0=expw, scalar=1.0, in1=mask_all[:, t, :],
                    op0=OP.mult, op1=OP.mult, accum_out=sumexp,
                )
                s["rsum"] = smalls.tile([P, 1], f32, tag="rsum", name="rsum")
                nc.vector.reciprocal(out=s["rsum"], in_=sumexp)
                diag = work.tile([P, K, P], bf16, tag="diag", name="diag")
                nc.vector.tensor_tensor(
                    out=diag,
                    in0=identb[:, None, :].broadcast_to([P, K, P]),
                    in1=s["w"][:, :, None].broadcast_to([P, K, P]),
                    op=OP.mult,
                )
                s["diag"] = diag

            def stage_c(t):
                # mcomb = sum_k w_k m_k (unnormalized); mcT via xbar transpose
                s = st[t]
                mc_ps = pmc.tile([P, D], f32, tag="mc", name="mc_ps")
                for k in range(K):
                    nc.tensor.matmul(
                        mc_ps, lhsT=s["diag"][:, k, :], rhs=s["qm"][:, 1 + k, :],
                        start=(k == 0), stop=(k == K - 1),
                    )
                mcb = work.tile([P, D], bf16, tag="mcb", name="mcb")
                nc.scalar.copy(out=mcb, in_=mc_ps)
                mcT = tload.tile([P, NCH, P], bf16, tag="mcT", name="mcT")
                nc.sync.dma_start_transpose(out=mcT, in_=mcb)
                s["mcT"] = mcT

            def stage_d(t):
                # mem = mcomb@Wvo ; m2 = [-mdot, mcomb.rowsum(Wvo)/D] ;
                # ge = exp(-rsum*mdot - qdot) ; rgp = sigmoid
                s = st[t]
                mem_ps = pmem.tile([P, D], f32, tag="mem", name="mem_ps")
                for c in range(NCH):
                    nc.tensor.matmul(
                        mem_ps, lhsT=s["mcT"][:, c, :], rhs=wvo_sb[:, c, :],
                        start=(c == 0), stop=(c == NCH - 1),
                    )
                m2_ps = pmix.tile([P, 2], f32, tag="mix2", name="m2_ps")
                for c in range(NCH):
                    nc.tensor.matmul(
                        m2_ps, lhsT=s["mcT"][:, c, :], rhs=gm_sb[:, c, :],
                        start=(c == 0), stop=(c == NCH - 1),
                    )
                s["mem_ps"] = mem_ps
                m2sb = smalls.tile([P, 2], f32, tag="m2sb", name="m2sb")
                nc.scalar.copy(out=m2sb, in_=m2_ps)
                s["memsum"] = m2sb[:, 1:2]
                ge = smalls.tile([P, 1], f32, tag="ge", name="ge")
                nc.scalar.activation(
                    out=ge, in_=m2sb[:, 0:1], func=AF.Exp,
                    bias=s["nqdot"], scale=s["rsum"],
                )
                gp1 = smalls.tile([P, 1], f32, tag="gp1", name="gp1")
                nc.scalar.activation(out=gp1, in_=ge, func=AF.Copy, bias=1.0)
                s["rgp"] = smalls.tile([P, 1], f32, tag="rgp", name="rgp")
                nc.vector.reciprocal(out=s["rgp"], in_=gp1)

            def stage_e1(t):
                # s = conf*rsum*sigmoid ; mem_s = s*mem (PSUM->SBUF bf16)
                s = st[t]
                s_sb = smalls.tile([P, 1], f32, tag="s_sb", name="s_sb")
                nc.vector.tensor_scalar(
                    out=s_sb, in0=s["rgp"], scalar1=conf_all[:, t:t + 1],
                    scalar2=s["rsum"], op0=OP.mult, op1=OP.mult,
                )
                s["s_sb"] = s_sb
                mem_s = work.tile([P, D], bf16, tag="mem_s", name="mem_s")
                nc.scalar.activation(
                    out=mem_s, in_=s["mem_ps"], func=AF.Copy, scale=s_sb
                )
                s["mem_s"] = mem_s

            def stage_e2(t):
                # out_pre = mem_s + q  (GPSIMD, bf16)
                s = st[t]
                out_pre = work.tile([P, D], bf16, tag="out_pre", name="out_pre")
                nc.gpsimd.tensor_tensor(
                    out=out_pre, in0=s["mem_s"], in1=s["qm"][:, 0, :], op=OP.add
                )
                s["out_pre"] = out_pre

            def stage_e3(t):
                # mu = qsum/D + s*memsum ; sumsq = sum(out_pre^2) ;
                # var = sumsq/D - mu^2
                s = st[t]
                mu = smalls.tile([P, 1], f32, tag="mu", name="mu")
                nc.vector.scalar_tensor_tensor(
                    out=mu, in0=s["memsum"], scalar=s["s_sb"], in1=s["qsum"],
                    op0=OP.mult, op1=OP.add,
                )
                s["mu"] = mu
                mu2 = smalls.tile([P, 1], f32, tag="mu2", name="mu2")
                nc.gpsimd.tensor_tensor(out=mu2, in0=mu, in1=mu, op=OP.mult)
                sumsq = smalls.tile([P, 1], f32, tag="sumsq", name="sumsq")
                sqscr = work.tile([P, D], bf16, tag="sqscr", name="sqscr")
                nc.scalar.activation(
                    out=sqscr, in_=s["out_pre"], func=AF.Square, accum_out=sumsq
                )
                varc = smalls.tile([P, 1], f32, tag="varc", name="varc")
                nc.vector.scalar_tensor_tensor(
                    out=varc, in0=sumsq, scalar=rD, in1=mu2,
                    op0=OP.mult, op1=OP.subtract,
                )
                s["varc"] = varc

            def stage_e4(t):
                # rstd = exp(-0.5 ln(var+eps)) ; out = (out_pre - mu)*rstd
                s = st.pop(t)
                lnv = smalls.tile([P, 1], f32, tag="lnv", name="lnv")
                nc.scalar.activation(
                    out=lnv, in_=s["varc"], func=AF.Ln, bias=epsc, scale=1.0
                )
                rstd = smalls.tile([P, 1], f32, tag="rstd", name="rstd")
                nc.scalar.activation(out=rstd, in_=lnv, func=AF.Exp, scale=-0.5)
                out_sb = work.tile([P, D], bf16, tag="out_sb", name="out_sb")
                nc.vector.tensor_scalar(
                    out=out_sb, in0=s["out_pre"], scalar1=s["mu"],
                    scalar2=rstd, op0=OP.subtract, op1=OP.mult,
                )
                nc.gpsimd.dma_start(out=o_t[t], in_=out_sb)

            stages = [
                (0, stage_in), (2, stage_a), (3, stage_b), (4, stage_c),
                (7, stage_d), (8, stage_e1), (9, stage_e2), (10, stage_e3),
                (11, stage_e4),
            ]
            total_lag = stages[-1][0]
            for i in range(ntiles + total_lag):
                for lag, fn in stages:
                    t = i - lag
                    if 0 <= t < ntiles:
                        fn(t)

    return nc


def _numpy_fallback(query, retrieved_memories, similarities, mask,
                    Wq, bq, Wk, bk, Wv, bv, Wo, bo, Wg, bg, ln_g, ln_b):
    x = query.astype(np.float64)
    m = retrieved_memories.astype(np.float64)
    q = x @ Wq + bq
    k = np.einsum("bkd,de->bke", m, Wk.astype(np.float64)) + bk
    v = np.einsum("bkd,de->bke", m, Wv.astype(np.float64)) + bv
    scores = np.einsum("bd,bkd->bk", q, k) * (D ** -0.5)
    scores = np.where(mask, scores, -np.inf)
    sm = scores - scores.max(-1, keepdims=True)
    w = np.exp(sm)
    w /= w.sum(-1, keepdims=True)
    w = np.where(mask, w, 0.0)
    mem = np.einsum("bk,bkd->bd", w, v) @ Wo + bo
    gate = 1 / (1 + np.exp(-(np.concatenate([x, mem], -1) @ Wg + bg)))
    conf = 1 / (1 + np.exp(-(similarities.max(-1, keepdims=True) - SIM_THRESH)))
    out = x + (gate * conf) * mem
    mu = out.mean(-1, keepdims=True)
    var = ((out - mu) ** 2).mean(-1, keepdims=True)
    out = (out - mu) / np.sqrt(var + LN_EPS) * ln_g + ln_b
    return out.astype(np.float32)


def kernel(**inputs):
    global LAST_RESULTS
    query = np.asarray(inputs["query"], dtype=np.float32)
    mem = np.asarray(inputs["retrieved_memories"], dtype=np.float32)
    sims = np.asarray(inputs["similarities"], dtype=np.float32)
    mask = np.asarray(inputs["mask"])
    Wq = np.asarray(inputs["Wq"], dtype=np.float64)
    Wk = np.asarray(inputs["Wk"], dtype=np.float64)
    Wv = np.asarray(inputs["Wv"], dtype=np.float64)
    Wo = np.asarray(inputs["Wo"], dtype=np.float64)
    Wg = np.asarray(inputs["Wg"], dtype=np.float64)

    # The device kernel folds all-zero biases / identity LN affine away.
    nontrivial = (
        any(np.any(np.asarray(inputs[n])) for n in ("bq", "bk", "bv", "bo", "bg"))
        or np.any(np.asarray(inputs["ln_b"]))
        or np.any(np.asarray(inputs["ln_g"]) != 1.0)
    )
    if nontrivial or query.shape != (B, D):
        return _numpy_fallback(
            query, mem, sims, mask, Wq=Wq, bq=np.asarray(inputs["bq"]),
            Wk=Wk, bk=np.asarray(inputs["bk"]), Wv=Wv, bv=np.asarray(inputs["bv"]),
            Wo=Wo, bo=np.asarray(inputs["bo"]), Wg=Wg, bg=np.asarray(inputs["bg"]),
            ln_g=np.asarray(inputs["ln_g"]), ln_b=np.asarray(inputs["ln_b"]),
        )

    import ml_dtypes
    bf = ml_dtypes.bfloat16
    wqk = np.ascontiguousarray(((Wq @ Wk.T) * (float(D) ** -0.5)).astype(bf))
    wvo64 = Wv @ Wo
    wvo = np.ascontiguousarray(wvo64.astype(bf))
    g1 = Wg[:D, 0]
    g2 = wvo64 @ Wg[D:, 0]
    ones_rD = np.full(D, 1.0 / D)
    gq = np.ascontiguousarray(np.stack([-g1, ones_rD], axis=1).astype(bf))
    gm = np.ascontiguousarray(
        np.stack([-g2, wvo64.sum(axis=1) / D], axis=1).astype(bf)
    )
    ident = np.eye(P, dtype=bf)

    if "nc" not in _CACHE:
        _CACHE["nc"] = _build()
    nc = _CACHE["nc"]

    qm = np.empty((B, (K + 1) * D), dtype=bf)
    qm[:, :D] = query
    qm[:, D:] = mem.reshape(B, K * D)
    maskf = np.ascontiguousarray(mask.astype(np.float32))
    conf = (1.0 / (1.0 + np.exp(-(sims.max(axis=-1) - SIM_THRESH)))).astype(
        np.float32
    ).reshape(B, 1)
    in_maps = []
    for c in range(N_CORES):
        sl = slice(c * ROWS, (c + 1) * ROWS)
        in_maps.append({
            "qm": qm[sl], "maskf": maskf[sl], "conf": conf[sl],
            "wqk": wqk, "wvo": wvo, "gq": gq, "gm": gm, "ident": ident,
        })

    from concourse.bass_utils import run_bass_kernel_spmd

    res = run_bass_kernel_spmd(nc, in_maps, list(range(N_CORES)), trace=TRACE)
    LAST_RESULTS = res
    return np.concatenate(
        [res.results[c]["o"].astype(np.float32) for c in range(N_CORES)], axis=0
    )


# revision 17
# speedup vs baseline: 1.1832x; 1.1832x over previous
"""Memory-augmented attention kernel for Trainium2 (Bass/Tile), 8-core data parallel.

Reference computation (per row b of B=32768, D=512, K=5):
    q' = query@Wq + bq
    k  = mem@Wk + bk ; v = mem@Wv + bv
    scores = (q'.k_j)/sqrt(D) masked-softmax -> w
    mem_out = (sum_j w_j v_j)@Wo + bo
    gate = sigmoid([query, mem_out]@Wg + bg); conf = sigmoid(max_sim - 0.7)
    out = LN(query + gate*conf*mem_out) * ln_g + ln_b

Algebraic refactoring (biases are zero / LN affine identity in this problem;
a numpy fallback covers the general case):
    scores_bk = m_bk . (query_b @ (Wq @ Wk^T) / sqrt(D))
    mem_out_b = (sum_k w_bk m_bk) @ (Wv @ Wo)
    gate_b    = sigmoid(query_b . Wg[:D] + mcomb_b . (Wv@Wo@Wg[D:]))
    conf      = sigmoid(max_k sims - 0.7)  (computed on host)

All bulk data moves HBM<->SBUF in bf16 (query+memories staged as one bf16
buffer, output stored bf16 and upcast on host), halving DMA traffic vs f32.
Scores are computed without max-subtraction (|scores| ~ N(0,1), exp safe).

Per 128-row tile, stages pipelined with deep lag so every engine streams:
    IN : DMA qm tile (bf16) + qT via SBUF->SBUF xbar dma_start_transpose
    A  : PE  pt = q@Wqk (lhsT=qT chunks), [-qdot, qsum/D] via gq cols;
         ACT copies pt (bf16), -qdot, qsum to SBUF
    B  : DVE scores dots (scalar_tensor_tensor bf16 + accum), ACT exp,
         DVE masked-sum STT -> w,sumexp, recip, 5x diag_k = I*w_k (bf16)
    C  : PE  mcomb = sum_k diag_k^T @ m_k (unnormalized); ACT copy bf16;
         DMA xbar transpose mcomb -> mcT
    D  : PE  mem = mcomb@Wvo, [-mdot, rowsum(Wvo)-dot] via gm cols;
         ACT ge = exp(-rsum*mdot - qdot) reading PSUM; DVE gp1, rgp
    E1 : DVE s = rgp*conf*rsum; ACT mem_s = s*mem (PSUM->SBUF bf16), memsum
    E2 : GPSIMD out_pre = mem_s + q (bf16)
    E3 : DVE mu; GPSIMD mu2; ACT sumsq (Square+accum); DVE var
    E4 : ACT lnv, rstd (exp(-0.5 ln(var+eps))); DVE nmr, apply (tensor_scalar
         2-AP-scalar, bf16 4x); GPSIMD out DMA (SWDGE)

This container's walrus build only encodes one sync-wait per instruction and
cannot encode EVENT_SEMAPHORE_RANGE_CLEAR; see _install_tile_patches.
"""

import numpy as np

B, D, K = 32768, 512, 5
N_CORES = 8
ROWS = B // N_CORES        # rows per core
P = 128                    # partitions
NT_FULL = ROWS // P        # tiles per core (32)
NCH = D // P               # 128-contraction chunks (4)
LN_EPS = 1e-5
SIM_THRESH = 0.7

_CACHE = {}

TRACE = False              # set by test harness to collect a HW profile
LAST_RESULTS = None        # BassKernelResults of the last run (for profiling)


def _install_tile_patches():
    """Work around two walrus limitations in this container:
    - instructions accept very few sync-wait slots: split the kernel-tail
      drain (which Tile loads with one wait per outstanding semaphore) into
      a chain of single-wait drains;
    - EVENT_SEMAPHORE_RANGE_CLEAR is not encodable: skip the on-device sem
      clear (each kernel() call executes a freshly loaded NEFF) while keeping
      the allocator bookkeeping.
    """
    import concourse.tile as tile
    from concourse.vector_clock import ScopedClock

    if getattr(tile.TileContext._drain_and_barrier, "_patched", False):
        return

    def patched(self, tick_clock, wait_clock):
        import bass_rust

        nc = self.nc
        drain_inst = nc.sync.drain()
        wait_clock.add_sem_waits(
            drain_inst.ins, ScopedClock({None: tick_clock.global_clock})
        )
        si = drain_inst.ins.sync_info
        waits = list(si.on_wait) if si is not None and si.on_wait else []
        if len(waits) > 1:
            drain_inst.ins.sync_info = bass_rust.SyncInfo(
                on_wait=waits[:1], on_update=list(si.on_update or [])
            )
            for w in waits[1:]:
                d2 = nc.sync.drain()
                d2.ins.sync_info = bass_rust.SyncInfo(on_wait=[w], on_update=[])
        nc.all_engine_barrier()
        assert self.sems is not None
        popped = nc._tile_sem_poison_stack.pop()
        assert popped is self._sem_poison
        sems = list(self.sems.allocated().values())
        sem_nums = [s.num for s in sems]
        nc._state.prepend_free_semaphores(sem_nums)
        for poison_set in nc._tile_sem_poison_stack:
            poison_set.update(sem_nums)
        nc.all_engine_barrier()

    patched._patched = True
    tile.TileContext._drain_and_barrier = patched

    # This walrus build accepts at most one sync-wait per instruction:
    # at commit time, peel off extra waits onto single-wait nop/drain
    # instructions inserted just before the owner.
    _orig_commit = tile.TileContext._commit_instruction

    def commit_patched(self, inst, lazy_reg_writes=True):
        import bass_rust
        from concourse import mybir

        si = inst.sync_info
        if si is not None and si.on_wait and len(si.on_wait) > 1:
            waits = list(si.on_wait)
            inst.sync_info = bass_rust.SyncInfo(
                on_wait=waits[-1:], on_update=list(si.on_update or [])
            )
            for w in waits[:-1]:
                eng = self.nc.engines[inst.engine]
                if not hasattr(eng, "engine_nop"):
                    nop = mybir.InstDrain(
                        name=self.nc.get_next_instruction_name(), ins=[], outs=[]
                    )
                    nop.engine = inst.engine
                else:
                    # sequencer-only ENGINE_NOP: carries the wait without
                    # flushing the compute pipeline the way a drain does
                    nop = eng.engine_nop().ins
                nop.sync_info = bass_rust.SyncInfo(on_wait=[w], on_update=[])
                self._add_instruction(nop)
        return _orig_commit(self, inst, lazy_reg_writes)

    tile.TileContext._commit_instruction = commit_patched


def _build(ntiles=NT_FULL):
    import concourse.bass as bass
    import concourse.tile as tile
    from concourse import mybir

    _install_tile_patches()

    f32 = mybir.dt.float32
    bf16 = mybir.dt.bfloat16
    AF = mybir.ActivationFunctionType
    OP = mybir.AluOpType

    rows = ntiles * P
    rD = 1.0 / float(D)

    nc = bass.Bass()
    qm_d = nc.declare_dram_parameter("qm", [rows, (K + 1) * D], bf16, isOutput=False)
    mask_d = nc.declare_dram_parameter("maskf", [rows, K], f32, isOutput=False)
    conf_d = nc.declare_dram_parameter("conf", [rows, 1], f32, isOutput=False)
    wqk_d = nc.declare_dram_parameter("wqk", [D, D], bf16, isOutput=False)
    wvo_d = nc.declare_dram_parameter("wvo", [D, D], bf16, isOutput=False)
    gq_d = nc.declare_dram_parameter("gq", [D, 2], bf16, isOutput=False)
    gm_d = nc.declare_dram_parameter("gm", [D, 2], bf16, isOutput=False)
    id_d = nc.declare_dram_parameter("ident", [P, P], bf16, isOutput=False)
    o_d = nc.declare_dram_parameter("o", [rows, D], bf16, isOutput=True)

    qm_t = qm_d.rearrange("(t p) x -> t p x", p=P)
    o_t = o_d.rearrange("(t p) d -> t p d", p=P)

    with tile.TileContext(nc) as tc:
        with (
            tc.tile_pool(name="consts", bufs=1) as consts,
            tc.tile_pool(name="qmload", bufs=12) as qmload,
            tc.tile_pool(name="tload", bufs=6) as tload,
            tc.tile_pool(name="work", bufs=4) as work,
            tc.tile_pool(name="smalls", bufs=12) as smalls,
            tc.tile_pool(name="ppt", bufs=2, space="PSUM") as ppt,
            tc.tile_pool(name="pmc", bufs=2, space="PSUM") as pmc,
            tc.tile_pool(name="pmem", bufs=2, space="PSUM") as pmem,
            tc.tile_pool(name="pmix", bufs=2, space="PSUM") as pmix,
        ):
            # ---- constants, loaded once ----
            wqk_sb = consts.tile([P, NCH, D], bf16)
            nc.sync.dma_start(out=wqk_sb, in_=wqk_d.rearrange("(c p) e -> p c e", p=P))
            wvo_sb = consts.tile([P, NCH, D], bf16)
            nc.sync.dma_start(out=wvo_sb, in_=wvo_d.rearrange("(c p) e -> p c e", p=P))
            gq_sb = consts.tile([P, NCH, 2], bf16)
            nc.sync.dma_start(out=gq_sb, in_=gq_d.rearrange("(c p) j -> p c j", p=P))
            gm_sb = consts.tile([P, NCH, 2], bf16)
            nc.sync.dma_start(out=gm_sb, in_=gm_d.rearrange("(c p) j -> p c j", p=P))
            identb = consts.tile([P, P], bf16)
            nc.sync.dma_start(out=identb, in_=id_d[:, :])
            mask_all = consts.tile([P, ntiles, K], f32)
            nc.sync.dma_start(
                out=mask_all, in_=mask_d.rearrange("(t p) k -> p t k", p=P)
            )
            conf_all = consts.tile([P, ntiles], f32)
            nc.sync.dma_start(
                out=conf_all, in_=conf_d.rearrange("(t p) j -> p (t j)", p=P)
            )
            epsc = consts.tile([P, 1], f32)
            nc.vector.memset(epsc, LN_EPS)

            def touch_dve(ap):
                tt = smalls.tile([P, 2], f32, tag="dvet", name="dvet")
                nc.vector.tensor_copy(out=tt[:, 0:ap.free_size()], in_=ap)

            def touch_act(ap):
                tt = smalls.tile([P, 2], f32, tag="actt", name="actt")
                nc.scalar.copy(out=tt[:, 0:ap.free_size()], in_=ap)

            def touch_gp(ap):
                tt = smalls.tile([P, 2], f32, tag="gpt", name="gpt")
                nc.gpsimd.tensor_copy(out=tt[:, 0:ap.free_size()], in_=ap)

            # Per-tile live state, keyed by tile index. Deep software pipeline
            # so each engine's in-order stream interleaves work from many
            # tiles instead of idling through each tile's dependency chain.
            st = {}

            def stage_in(t):
                s = st.setdefault(t, {})
                qm = qmload.tile([P, K + 1, D], bf16, tag="qm", name="qmtile")
                nc.sync.dma_start(out=qm, in_=qm_t[t].rearrange("p (s d) -> p s d", d=D))
                qT = tload.tile([P, NCH, P], bf16, tag="qT", name="qT")
                nc.sync.dma_start_transpose(out=qT, in_=qm[:, 0, :])
                s["qm"] = qm
                s["qT"] = qT

            def stage_a(t):
                # pt = q@Wqk ; q2 = [-qdot, qsum/D]
                s = st[t]
                pt_ps = ppt.tile([P, D], f32, tag="pt", name="pt_ps")
                for c in range(NCH):
                    nc.tensor.matmul(
                        pt_ps, lhsT=s["qT"][:, c, :], rhs=wqk_sb[:, c, :],
                        start=(c == 0), stop=(c == NCH - 1),
                    )
                q2_ps = pmix.tile([P, 2], f32, tag="mix2", name="q2_ps")
                for c in range(NCH):
                    nc.tensor.matmul(
                        q2_ps, lhsT=s["qT"][:, c, :], rhs=gq_sb[:, c, :],
                        start=(c == 0), stop=(c == NCH - 1),
                    )
                s["pt"] = work.tile([P, D], bf16, tag="pt_sb", name="pt_sb")
                nc.scalar.copy(out=s["pt"], in_=pt_ps)
                q2sb = smalls.tile([P, 2], f32, tag="q2sb", name="q2sb")
                nc.scalar.copy(out=q2sb, in_=q2_ps)
                s["nqdot"] = q2sb[:, 0:1]
                s["qsum"] = q2sb[:, 1:2]

            def stage_b(t):
                # raw_k = m_k . pt ; w = exp(raw)*mask ; rsum = 1/sum(w)
                s = st[t]
                raw = smalls.tile([P, K], f32, tag="raw", name="raw")
                scratch = work.tile([P, D], bf16, tag="scratch", name="scratch")
                touch_dve(s["qm"][:, 1, 0:2])
                touch_dve(s["pt"][:, 0:2])
                for k in range(K):
                    nc.vector.scalar_tensor_tensor(
                        out=scratch, in0=s["qm"][:, 1 + k, :], scalar=1.0,
                        in1=s["pt"], op0=OP.mult, op1=OP.mult,
                        accum_out=raw[:, k:k + 1],
                    )
                expw = smalls.tile([P, K], f32, tag="expw", name="expw")
                nc.scalar.activation(out=expw, in_=raw, func=AF.Exp)
                s["w"] = smalls.tile([P, K], bf16, tag="w", name="w")
                sumexp = smalls.tile([P, 1], f32, tag="sumexp", name="sumexp")
                nc.vector.scalar_tensor_tensor(
                    out=s["w"], in0=expw, scalar=1.0, in1=mask_all[:, t, :],
                    op0=OP.mult, op1=OP.mult, accum_out=sumexp,
                )
                s["rsum"] = smalls.tile([P, 1], f32, tag="rsum", name="rsum")
                nc.vector.reciprocal(out=s["rsum"], in_=sumexp)
                diag = work.tile([P, K, P], bf16, tag="diag", name="diag")
                nc.vector.tensor_tensor(
                    out=diag,
                    in0=identb[:, None, :].broadcast_to([P, K, P]),
                    in1=s["w"][:, :, None].broadcast_to([P, K, P]),
                    op=OP.mult,
                )
                s["diag"] = diag

            def stage_c(t):
                # mcomb = sum_k w_k m_k (unnormalized); mcT via xbar transpose
                s = st[t]
                mc_ps = pmc.tile([P, D], f32, tag="mc", name="mc_ps")
                for k in range(K):
                    nc.tensor.matmul(
                        mc_ps, lhsT=s["diag"][:, k, :], rhs=s["qm"][:, 1 + k, :],
                        start=(k == 0), stop=(k == K - 1),
                    )
                mcb = work.tile([P, D], bf16, tag="mcb", name="mcb")
                nc.scalar.copy(out=mcb, in_=mc_ps)
                mcT = tload.tile([P, NCH, P], bf16, tag="mcT", name="mcT")
                nc.sync.dma_start_transpose(out=mcT, in_=mcb)
                s["mcT"] = mcT

            def stage_d(t):
                # mem = mcomb@Wvo ; m2 = [-mdot, mcomb.rowsum(Wvo)/D] ;
                # ge = exp(-rsum*mdot - qdot) ; rgp = sigmoid
                s = st[t]
                mem_ps = pmem.tile([P, D], f32, tag="mem", name="mem_ps")
                for c in range(NCH):
                    nc.tensor.matmul(
                        mem_ps, lhsT=s["mcT"][:, c, :], rhs=wvo_sb[:, c, :],
                        start=(c == 0), stop=(c == NCH - 1),
                    )
                m2_ps = pmix.tile([P, 2], f32, tag="mix2", name="m2_ps")
                for c in range(NCH):
                    nc.tensor.matmul(
                        m2_ps, lhsT=s["mcT"][:, c, :], rhs=gm_sb[:, c, :],
                        start=(c == 0), stop=(c == NCH - 1),
                    )
                s["mem_ps"] = mem_ps
                m2sb = smalls.tile([P, 2], f32, tag="m2sb", name="m2sb")
                nc.scalar.copy(out=m2sb, in_=m2_ps)
                s["memsum"] = m2sb[:, 1:2]
                ge = smalls.tile([P, 1], f32, tag="ge", name="ge")
                touch_act(s["rsum"][:, 0:1])
                nc.scalar.activation(
                    out=ge, in_=m2sb[:, 0:1], func=AF.Exp,
                    bias=s["nqdot"], scale=s["rsum"],
                )
                gp1 = smalls.tile([P, 1], f32, tag="gp1", name="gp1")
                nc.vector.tensor_scalar(
                    out=gp1, in0=ge, scalar1=1.0, scalar2=None, op0=OP.add
                )
                s["rgp"] = smalls.tile([P, 1], f32, tag="rgp", name="rgp")
                nc.vector.reciprocal(out=s["rgp"], in_=gp1)

            def stage_e1(t):
                # s = conf*rsum*sigmoid ; mem_s = s*mem (PSUM->SBUF bf16)
                s = st[t]
                s_sb = smalls.tile([P, 1], f32, tag="s_sb", name="s_sb")
                nc.vector.tensor_scalar(
                    out=s_sb, in0=s["rgp"], scalar1=conf_all[:, t:t + 1],
                    scalar2=s["rsum"], op0=OP.mult, op1=OP.mult,
                )
                s["s_sb"] = s_sb
                mem_s = work.tile([P, D], bf16, tag="mem_s", name="mem_s")
                touch_act(s_sb[:, 0:1])
                nc.scalar.activation(
                    out=mem_s, in_=s["mem_ps"], func=AF.Copy, scale=s_sb
                )
                s["mem_s"] = mem_s

            def stage_e2(t):
                # out_pre = mem_s + q  (GPSIMD, bf16)
                s = st[t]
                out_pre = work.tile([P, D], bf16, tag="out_pre", name="out_pre")
                touch_gp(s["mem_s"][:, 0:2])
                nc.gpsimd.tensor_tensor(
                    out=out_pre, in0=s["mem_s"], in1=s["qm"][:, 0, :], op=OP.add
                )
                s["out_pre"] = out_pre

            def stage_e3(t):
                # mu = qsum/D + s*memsum ; sumsq = sum(out_pre^2) ;
                # var = sumsq/D - mu^2
                s = st[t]
                mu = smalls.tile([P, 1], f32, tag="mu", name="mu")
                nc.vector.scalar_tensor_tensor(
                    out=mu, in0=s["memsum"], scalar=s["s_sb"], in1=s["qsum"],
                    op0=OP.mult, op1=OP.add,
                )
                s["mu"] = mu
                mu2 = smalls.tile([P, 1], f32, tag="mu2", name="mu2")
                nc.gpsimd.tensor_tensor(out=mu2, in0=mu, in1=mu, op=OP.mult)
                sumsq = smalls.tile([P, 1], f32, tag="sumsq", name="sumsq")
                sqscr = work.tile([P, D], bf16, tag="sqscr", name="sqscr")
                touch_act(s["out_pre"][:, 0:2])
                nc.scalar.activation(
                    out=sqscr, in_=s["out_pre"], func=AF.Square, accum_out=sumsq
                )
                varc = smalls.tile([P, 1], f32, tag="varc", name="varc")
                nc.vector.scalar_tensor_tensor(
                    out=varc, in0=sumsq, scalar=rD, in1=mu2,
                    op0=OP.mult, op1=OP.subtract,
                )
                s["varc"] = varc

            def stage_e4(t):
                # rstd = exp(-0.5 ln(var+eps)) ; out = (out_pre - mu)*rstd
                s = st.pop(t)
                lnv = smalls.tile([P, 1], f32, tag="lnv", name="lnv")
                nc.scalar.activation(
                    out=lnv, in_=s["varc"], func=AF.Ln, bias=epsc, scale=1.0
                )
                rstd = smalls.tile([P, 1], f32, tag="rstd", name="rstd")
                nc.scalar.activation(out=rstd, in_=lnv, func=AF.Exp, scale=-0.5)
                nmr = smalls.tile([P, 1], f32, tag="nmr", name="nmr")
                nc.vector.tensor_scalar(
                    out=nmr, in0=s["mu"], scalar1=rstd, scalar2=-1.0,
                    op0=OP.mult, op1=OP.mult,
                )
                out_sb = work.tile([P, D], bf16, tag="out_sb", name="out_sb")
                nc.vector.tensor_scalar(
                    out=out_sb, in0=s["out_pre"], scalar1=rstd,
                    scalar2=nmr, op0=OP.mult, op1=OP.add,
                )
                touch_gp(out_sb[:, 0:2])
                nc.gpsimd.dma_start(out=o_t[t], in_=out_sb)

            stages = [
                (0, stage_in), (2, stage_a), (3, stage_b), (4, stage_c),
                (7, stage_d), (8, stage_e1), (9, stage_e2), (10, stage_e3),
                (11, stage_e4),
            ]
            total_lag = stages[-1][0]
            for i in range(ntiles + total_lag):
                for lag, fn in stages:
                    t = i - lag
                    if 0 <= t < ntiles:
                        fn(t)

    return nc


def _numpy_fallback(query, retrieved_memories, similarities, mask,
                    Wq, bq, Wk, bk, Wv, bv, Wo, bo, Wg, bg, ln_g, ln_b):
    x = query.astype(np.float64)
    m = retrieved_memories.astype(np.float64)
    q = x @ Wq + bq
    k = np.einsum("bkd,de->bke", m, Wk.astype(np.float64)) + bk
    v = np.einsum("bkd,de->bke", m, Wv.astype(np.float64)) + bv
    scores = np.einsum("bd,bkd->bk", q, k) * (D ** -0.5)
    scores = np.where(mask, scores, -np.inf)
    sm = scores - scores.max(-1, keepdims=True)
    w = np.exp(sm)
    w /= w.sum(-1, keepdims=True)
    w = np.where(mask, w, 0.0)
    mem = np.einsum("bk,bkd->bd", w, v) @ Wo + bo
    gate = 1 / (1 + np.exp(-(np.concatenate([x, mem], -1) @ Wg + bg)))
    conf = 1 / (1 + np.exp(-(similarities.max(-1, keepdims=True) - SIM_THRESH)))
    out = x + (gate * conf) * mem
    mu = out.mean(-1, keepdims=True)
    var = ((out - mu) ** 2).mean(-1, keepdims=True)
    out = (out - mu) / np.sqrt(var + LN_EPS) * ln_g + ln_b
    return out.astype(np.float32)


def kernel(**inputs):
    global LAST_RESULTS
    query = np.asarray(inputs["query"], dtype=np.float32)
    mem = np.asarray(inputs["retrieved_memories"], dtype=np.float32)
    sims = np.asarray(inputs["similarities"], dtype=np.float32)
    mask = np.asarray(inputs["mask"])
    Wq = np.asarray(inputs["Wq"], dtype=np.float64)
    Wk = np.asarray(inputs["Wk"], dtype=np.float64)
    Wv = np.asarray(inputs["Wv"], dtype=np.float64)
    Wo = np.asarray(inputs["Wo"], dtype=np.float64)
    Wg = np.asarray(inputs["Wg"], dtype=np.float64)

    # The device kernel folds all-zero biases / identity LN affine away.
    nontrivial = (
        any(np.any(np.asarray(inputs[n])) for n in ("bq", "bk", "bv", "bo", "bg"))
        or np.any(np.asarray(inputs["ln_b"]))
        or np.any(np.asarray(inputs["ln_g"]) != 1.0)
    )
    if nontrivial or query.shape != (B, D):
        return _numpy_fallback(
            query, mem, sims, mask, Wq=Wq, bq=np.asarray(inputs["bq"]),
            Wk=Wk, bk=np.asarray(inputs["bk"]), Wv=Wv, bv=np.asarray(inputs["bv"]),
            Wo=Wo, bo=np.asarray(inputs["bo"]), Wg=Wg, bg=np.asarray(inputs["bg"]),
            ln_g=np.asarray(inputs["ln_g"]), ln_b=np.asarray(inputs["ln_b"]),
        )

    import ml_dtypes
    bf = ml_dtypes.bfloat16
    wqk = np.ascontiguousarray(((Wq @ Wk.T) * (float(D) ** -0.5)).astype(bf))
    wvo64 = Wv @ Wo
    wvo = np.ascontiguousarray(wvo64.astype(bf))
    g1 = Wg[:D, 0]
    g2 = wvo64 @ Wg[D:, 0]
    ones_rD = np.full(D, 1.0 / D)
    gq = np.ascontiguousarray(np.stack([-g1, ones_rD], axis=1).astype(bf))
    gm = np.ascontiguousarray(
        np.stack([-g2, wvo64.sum(axis=1) / D], axis=1).astype(bf)
    )
    ident = np.eye(P, dtype=bf)

    if "nc" not in _CACHE:
        _CACHE["nc"] = _build()
    nc = _CACHE["nc"]

    qm = np.empty((B, (K + 1) * D), dtype=bf)
    qm[:, :D] = query
    qm[:, D:] = mem.reshape(B, K * D)
    maskf = np.ascontiguousarray(mask.astype(np.float32))
    conf = (1.0 / (1.0 + np.exp(-(sims.max(axis=-1) - SIM_THRESH)))).astype(
        np.float32
    ).reshape(B, 1)
    in_maps = []
    for c in range(N_CORES):
        sl = slice(c * ROWS, (c + 1) * ROWS)
        in_maps.append({
            "qm": qm[sl], "maskf": maskf[sl], "conf": conf[sl],
            "wqk": wqk, "wvo": wvo, "gq": gq, "gm": gm, "ident": ident,
        })

    from concourse.bass_utils import run_bass_kernel_spmd

    res = run_bass_kernel_spmd(nc, in_maps, list(range(N_CORES)), trace=TRACE)
    LAST_RESULTS = res
    return np.concatenate(
        [res.results[c]["o"].astype(np.float32) for c in range(N_CORES)], axis=0
    )


# revision 18
# speedup vs baseline: 1.1867x; 1.0030x over previous
"""Memory-augmented attention kernel for Trainium2 (Bass/Tile), 8-core data parallel.

Reference computation (per row b of B=32768, D=512, K=5):
    q' = query@Wq + bq
    k  = mem@Wk + bk ; v = mem@Wv + bv
    scores = (q'.k_j)/sqrt(D) masked-softmax -> w
    mem_out = (sum_j w_j v_j)@Wo + bo
    gate = sigmoid([query, mem_out]@Wg + bg); conf = sigmoid(max_sim - 0.7)
    out = LN(query + gate*conf*mem_out) * ln_g + ln_b

Algebraic refactoring (biases are zero / LN affine identity in this problem;
a numpy fallback covers the general case):
    scores_bk = m_bk . (query_b @ (Wq @ Wk^T) / sqrt(D))
    mem_out_b = (sum_k w_bk m_bk) @ (Wv @ Wo)
    gate_b    = sigmoid(query_b . Wg[:D] + mcomb_b . (Wv@Wo@Wg[D:]))
    conf      = sigmoid(max_k sims - 0.7)  (computed on host)

All bulk data moves HBM<->SBUF in bf16 (query+memories staged as one bf16
buffer, output stored bf16 and upcast on host), halving DMA traffic vs f32.
Scores are computed without max-subtraction (|scores| ~ N(0,1), exp safe).

Per 128-row tile, stages pipelined with deep lag so every engine streams:
    IN : DMA qm tile (bf16) + qT via SBUF->SBUF xbar dma_start_transpose
    A  : PE  pt = q@Wqk (lhsT=qT chunks), [-qdot, qsum/D] via gq cols;
         ACT copies pt (bf16), -qdot, qsum to SBUF
    B  : DVE scores dots (scalar_tensor_tensor bf16 + accum), ACT exp,
         DVE masked-sum STT -> w,sumexp, recip, 5x diag_k = I*w_k (bf16)
    C  : PE  mcomb = sum_k diag_k^T @ m_k (unnormalized); ACT copy bf16;
         DMA xbar transpose mcomb -> mcT
    D  : PE  mem = mcomb@Wvo, [-mdot, rowsum(Wvo)-dot] via gm cols;
         ACT ge = exp(-rsum*mdot - qdot) reading PSUM; DVE gp1, rgp
    E1 : DVE s = rgp*conf*rsum; ACT mem_s = s*mem (PSUM->SBUF bf16), memsum
    E2 : GPSIMD out_pre = mem_s + q (bf16)
    E3 : DVE mu; GPSIMD mu2; ACT sumsq (Square+accum); DVE var
    E4 : ACT lnv, rstd (exp(-0.5 ln(var+eps))); DVE nmr, apply (tensor_scalar
         2-AP-scalar, bf16 4x); GPSIMD out DMA (SWDGE)

This container's walrus build only encodes one sync-wait per instruction and
cannot encode EVENT_SEMAPHORE_RANGE_CLEAR; see _install_tile_patches.
"""

import numpy as np

B, D, K = 32768, 512, 5
N_CORES = 8
ROWS = B // N_CORES        # rows per core
P = 128                    # partitions
NT_FULL = ROWS // P        # tiles per core (32)
NCH = D // P               # 128-contraction chunks (4)
LN_EPS = 1e-5
SIM_THRESH = 0.7

_CACHE = {}

TRACE = False              # set by test harness to collect a HW profile
LAST_RESULTS = None        # BassKernelResults of the last run (for profiling)


def _install_tile_patches():
    """Work around two walrus limitations in this container:
    - instructions accept very few sync-wait slots: split the kernel-tail
      drain (which Tile loads with one wait per outstanding semaphore) into
      a chain of single-wait drains;
    - EVENT_SEMAPHORE_RANGE_CLEAR is not encodable: skip the on-device sem
      clear (each kernel() call executes a freshly loaded NEFF) while keeping
      the allocator bookkeeping.
    """
    import concourse.tile as tile
    from concourse.vector_clock import ScopedClock

    if getattr(tile.TileContext._drain_and_barrier, "_patched", False):
        return

    def patched(self, tick_clock, wait_clock):
        import bass_rust

        nc = self.nc
        drain_inst = nc.sync.drain()
        wait_clock.add_sem_waits(
            drain_inst.ins, ScopedClock({None: tick_clock.global_clock})
        )
        si = drain_inst.ins.sync_info
        waits = list(si.on_wait) if si is not None and si.on_wait else []
        if len(waits) > 1:
            drain_inst.ins.sync_info = bass_rust.SyncInfo(
                on_wait=waits[:1], on_update=list(si.on_update or [])
            )
            for w in waits[1:]:
                d2 = nc.sync.drain()
                d2.ins.sync_info = bass_rust.SyncInfo(on_wait=[w], on_update=[])
        nc.all_engine_barrier()
        assert self.sems is not None
        popped = nc._tile_sem_poison_stack.pop()
        assert popped is self._sem_poison
        sems = list(self.sems.allocated().values())
        sem_nums = [s.num for s in sems]
        nc._state.prepend_free_semaphores(sem_nums)
        for poison_set in nc._tile_sem_poison_stack:
            poison_set.update(sem_nums)
        nc.all_engine_barrier()

    patched._patched = True
    tile.TileContext._drain_and_barrier = patched

    # This walrus build accepts at most one sync-wait per instruction:
    # at commit time, peel off extra waits onto single-wait nop/drain
    # instructions inserted just before the owner.
    _orig_commit = tile.TileContext._commit_instruction

    def commit_patched(self, inst, lazy_reg_writes=True):
        import bass_rust
        from concourse import mybir

        si = inst.sync_info
        if si is not None and si.on_wait and len(si.on_wait) > 1:
            waits = list(si.on_wait)
            inst.sync_info = bass_rust.SyncInfo(
                on_wait=waits[-1:], on_update=list(si.on_update or [])
            )
            for w in waits[:-1]:
                eng = self.nc.engines[inst.engine]
                if not hasattr(eng, "engine_nop"):
                    nop = mybir.InstDrain(
                        name=self.nc.get_next_instruction_name(), ins=[], outs=[]
                    )
                    nop.engine = inst.engine
                else:
                    # sequencer-only ENGINE_NOP: carries the wait without
                    # flushing the compute pipeline the way a drain does
                    nop = eng.engine_nop().ins
                nop.sync_info = bass_rust.SyncInfo(on_wait=[w], on_update=[])
                self._add_instruction(nop)
        return _orig_commit(self, inst, lazy_reg_writes)

    tile.TileContext._commit_instruction = commit_patched


def _build(ntiles=NT_FULL):
    import concourse.bass as bass
    import concourse.tile as tile
    from concourse import mybir

    _install_tile_patches()

    f32 = mybir.dt.float32
    bf16 = mybir.dt.bfloat16
    AF = mybir.ActivationFunctionType
    OP = mybir.AluOpType

    rows = ntiles * P
    rD = 1.0 / float(D)

    nc = bass.Bass()
    qm_d = nc.declare_dram_parameter("qm", [rows, (K + 1) * D], bf16, isOutput=False)
    mask_d = nc.declare_dram_parameter("maskf", [rows, K], f32, isOutput=False)
    conf_d = nc.declare_dram_parameter("conf", [rows, 1], f32, isOutput=False)
    wqk_d = nc.declare_dram_parameter("wqk", [D, D], bf16, isOutput=False)
    wvo_d = nc.declare_dram_parameter("wvo", [D, D], bf16, isOutput=False)
    gq_d = nc.declare_dram_parameter("gq", [D, 2], bf16, isOutput=False)
    gm_d = nc.declare_dram_parameter("gm", [D, 2], bf16, isOutput=False)
    id_d = nc.declare_dram_parameter("ident", [P, P], bf16, isOutput=False)
    o_d = nc.declare_dram_parameter("o", [rows, D], bf16, isOutput=True)

    qm_t = qm_d.rearrange("(t p) x -> t p x", p=P)
    o_t = o_d.rearrange("(t p) d -> t p d", p=P)

    with tile.TileContext(nc) as tc:
        with (
            tc.tile_pool(name="consts", bufs=1) as consts,
            tc.tile_pool(name="qmload", bufs=12) as qmload,
            tc.tile_pool(name="tload", bufs=6) as tload,
            tc.tile_pool(name="work", bufs=5) as work,
            tc.tile_pool(name="smalls", bufs=12) as smalls,
            tc.tile_pool(name="ppt", bufs=2, space="PSUM") as ppt,
            tc.tile_pool(name="pmc", bufs=2, space="PSUM") as pmc,
            tc.tile_pool(name="pmem", bufs=2, space="PSUM") as pmem,
            tc.tile_pool(name="pmix", bufs=2, space="PSUM") as pmix,
        ):
            # ---- constants, loaded once ----
            wqk_sb = consts.tile([P, NCH, D], bf16)
            nc.sync.dma_start(out=wqk_sb, in_=wqk_d.rearrange("(c p) e -> p c e", p=P))
            wvo_sb = consts.tile([P, NCH, D], bf16)
            nc.sync.dma_start(out=wvo_sb, in_=wvo_d.rearrange("(c p) e -> p c e", p=P))
            gq_sb = consts.tile([P, NCH, 2], bf16)
            nc.sync.dma_start(out=gq_sb, in_=gq_d.rearrange("(c p) j -> p c j", p=P))
            gm_sb = consts.tile([P, NCH, 2], bf16)
            nc.sync.dma_start(out=gm_sb, in_=gm_d.rearrange("(c p) j -> p c j", p=P))
            identb = consts.tile([P, P], bf16)
            nc.sync.dma_start(out=identb, in_=id_d[:, :])
            mask_all = consts.tile([P, ntiles, K], f32)
            nc.sync.dma_start(
                out=mask_all, in_=mask_d.rearrange("(t p) k -> p t k", p=P)
            )
            conf_all = consts.tile([P, ntiles], f32)
            nc.sync.dma_start(
                out=conf_all, in_=conf_d.rearrange("(t p) j -> p (t j)", p=P)
            )
            epsc = consts.tile([P, 1], f32)
            nc.vector.memset(epsc, LN_EPS)

            def touch_dve(ap):
                tt = smalls.tile([P, 2], f32, tag="dvet", name="dvet")
                nc.vector.tensor_copy(out=tt[:, 0:ap.free_size()], in_=ap)

            def touch_act(ap):
                tt = smalls.tile([P, 2], f32, tag="actt", name="actt")
                nc.scalar.copy(out=tt[:, 0:ap.free_size()], in_=ap)

            def touch_gp(ap):
                tt = smalls.tile([P, 2], f32, tag="gpt", name="gpt")
                nc.gpsimd.tensor_copy(out=tt[:, 0:ap.free_size()], in_=ap)

            # Per-tile live state, keyed by tile index. Deep software pipeline
            # so each engine's in-order stream interleaves work from many
            # tiles instead of idling through each tile's dependency chain.
            st = {}

            def stage_in(t):
                s = st.setdefault(t, {})
                qm = qmload.tile([P, K + 1, D], bf16, tag="qm", name="qmtile")
                nc.sync.dma_start(out=qm, in_=qm_t[t].rearrange("p (s d) -> p s d", d=D))
                qT = tload.tile([P, NCH, P], bf16, tag="qT", name="qT")
                nc.sync.dma_start_transpose(out=qT, in_=qm[:, 0, :])
                s["qm"] = qm
                s["qT"] = qT

            def stage_a(t):
                # pt = q@Wqk ; q2 = [-qdot, qsum/D]
                s = st[t]
                pt_ps = ppt.tile([P, D], f32, tag="pt", name="pt_ps")
                for c in range(NCH):
                    nc.tensor.matmul(
                        pt_ps, lhsT=s["qT"][:, c, :], rhs=wqk_sb[:, c, :],
                        start=(c == 0), stop=(c == NCH - 1),
                    )
                q2_ps = pmix.tile([P, 2], f32, tag="mix2", name="q2_ps")
                for c in range(NCH):
                    nc.tensor.matmul(
                        q2_ps, lhsT=s["qT"][:, c, :], rhs=gq_sb[:, c, :],
                        start=(c == 0), stop=(c == NCH - 1),
                    )
                s["pt"] = work.tile([P, D], bf16, tag="pt_sb", name="pt_sb")
                nc.scalar.copy(out=s["pt"], in_=pt_ps)
                q2sb = smalls.tile([P, 2], f32, tag="q2sb", name="q2sb")
                nc.scalar.copy(out=q2sb, in_=q2_ps)
                s["nqdot"] = q2sb[:, 0:1]
                s["qsum"] = q2sb[:, 1:2]

            def stage_b1(t):
                # raw_k = m_k . pt ; expw = exp(raw)
                s = st[t]
                raw = smalls.tile([P, K], f32, tag="raw", name="raw")
                scratch = work.tile([P, D], bf16, tag="scratch", name="scratch")
                touch_dve(s["qm"][:, 1, 0:2])
                touch_dve(s["pt"][:, 0:2])
                for k in range(K):
                    nc.vector.scalar_tensor_tensor(
                        out=scratch, in0=s["qm"][:, 1 + k, :], scalar=1.0,
                        in1=s["pt"], op0=OP.mult, op1=OP.mult,
                        accum_out=raw[:, k:k + 1],
                    )
                expw = smalls.tile([P, K], f32, tag="expw", name="expw")
                nc.scalar.activation(out=expw, in_=raw, func=AF.Exp)
                s["expw"] = expw

            def stage_b2(t):
                # w = expw*mask ; rsum ; diag_k = I*w_k
                s = st[t]
                expw = s.pop("expw")
                s["w"] = smalls.tile([P, K], bf16, tag="w", name="w")
                sumexp = smalls.tile([P, 1], f32, tag="sumexp", name="sumexp")
                nc.vector.scalar_tensor_tensor(
                    out=s["w"], in0=expw, scalar=1.0, in1=mask_all[:, t, :],
                    op0=OP.mult, op1=OP.mult, accum_out=sumexp,
                )
                s["rsum"] = smalls.tile([P, 1], f32, tag="rsum", name="rsum")
                nc.vector.reciprocal(out=s["rsum"], in_=sumexp)
                diag = work.tile([P, K, P], bf16, tag="diag", name="diag")
                nc.vector.tensor_tensor(
                    out=diag,
                    in0=identb[:, None, :].broadcast_to([P, K, P]),
                    in1=s["w"][:, :, None].broadcast_to([P, K, P]),
                    op=OP.mult,
                )
                s["diag"] = diag

            def stage_c(t):
                # mcomb = sum_k w_k m_k (unnormalized); mcT via xbar transpose
                s = st[t]
                mc_ps = pmc.tile([P, D], f32, tag="mc", name="mc_ps")
                for k in range(K):
                    nc.tensor.matmul(
                        mc_ps, lhsT=s["diag"][:, k, :], rhs=s["qm"][:, 1 + k, :],
                        start=(k == 0), stop=(k == K - 1),
                    )
                mcb = work.tile([P, D], bf16, tag="mcb", name="mcb")
                nc.scalar.copy(out=mcb, in_=mc_ps)
                mcT = tload.tile([P, NCH, P], bf16, tag="mcT", name="mcT")
                nc.sync.dma_start_transpose(out=mcT, in_=mcb)
                s["mcT"] = mcT

            def stage_d(t):
                # mem = mcomb@Wvo ; m2 = [-mdot, mcomb.rowsum(Wvo)/D] ;
                # ge = exp(-rsum*mdot - qdot) ; rgp = sigmoid
                s = st[t]
                mem_ps = pmem.tile([P, D], f32, tag="mem", name="mem_ps")
                for c in range(NCH):
                    nc.tensor.matmul(
                        mem_ps, lhsT=s["mcT"][:, c, :], rhs=wvo_sb[:, c, :],
                        start=(c == 0), stop=(c == NCH - 1),
                    )
                m2_ps = pmix.tile([P, 2], f32, tag="mix2", name="m2_ps")
                for c in range(NCH):
                    nc.tensor.matmul(
                        m2_ps, lhsT=s["mcT"][:, c, :], rhs=gm_sb[:, c, :],
                        start=(c == 0), stop=(c == NCH - 1),
                    )
                s["mem_ps"] = mem_ps
                m2sb = smalls.tile([P, 2], f32, tag="m2sb", name="m2sb")
                nc.scalar.copy(out=m2sb, in_=m2_ps)
                s["memsum"] = m2sb[:, 1:2]
                ge = smalls.tile([P, 1], f32, tag="ge", name="ge")
                touch_act(s["rsum"][:, 0:1])
                nc.scalar.activation(
                    out=ge, in_=m2sb[:, 0:1], func=AF.Exp,
                    bias=s["nqdot"], scale=s["rsum"],
                )
                s["ge"] = ge

            def stage_d2e1(t):
                # sigmoid chain ; s = conf*rsum*sigmoid ; mem_s = s*mem
                s = st[t]
                gp1 = smalls.tile([P, 1], f32, tag="gp1", name="gp1")
                nc.vector.tensor_scalar(
                    out=gp1, in0=s.pop("ge"), scalar1=1.0, scalar2=None, op0=OP.add
                )
                rgp = smalls.tile([P, 1], f32, tag="rgp", name="rgp")
                nc.vector.reciprocal(out=rgp, in_=gp1)
                s["rgp"] = rgp
                s_sb = smalls.tile([P, 1], f32, tag="s_sb", name="s_sb")
                nc.vector.tensor_scalar(
                    out=s_sb, in0=s["rgp"], scalar1=conf_all[:, t:t + 1],
                    scalar2=s["rsum"], op0=OP.mult, op1=OP.mult,
                )
                s["s_sb"] = s_sb
                mem_s = work.tile([P, D], bf16, tag="mem_s", name="mem_s")
                touch_act(s_sb[:, 0:1])
                nc.scalar.activation(
                    out=mem_s, in_=s["mem_ps"], func=AF.Copy, scale=s_sb
                )
                s["mem_s"] = mem_s

            def stage_e2(t):
                # out_pre = mem_s + q  (GPSIMD, bf16)
                s = st[t]
                out_pre = work.tile([P, D], bf16, tag="out_pre", name="out_pre")
                touch_gp(s["mem_s"][:, 0:2])
                nc.gpsimd.tensor_tensor(
                    out=out_pre, in0=s["mem_s"], in1=s["qm"][:, 0, :], op=OP.add
                )
                s["out_pre"] = out_pre

            def stage_e3(t):
                # mu = qsum/D + s*memsum ; sumsq = sum(out_pre^2) ;
                # var = sumsq/D - mu^2
                s = st[t]
                mu = smalls.tile([P, 1], f32, tag="mu", name="mu")
                nc.vector.scalar_tensor_tensor(
                    out=mu, in0=s["memsum"], scalar=s["s_sb"], in1=s["qsum"],
                    op0=OP.mult, op1=OP.add,
                )
                s["mu"] = mu
                mu2 = smalls.tile([P, 1], f32, tag="mu2", name="mu2")
                nc.gpsimd.tensor_tensor(out=mu2, in0=mu, in1=mu, op=OP.mult)
                sumsq = smalls.tile([P, 1], f32, tag="sumsq", name="sumsq")
                sqscr = work.tile([P, D], bf16, tag="sqscr", name="sqscr")
                touch_act(s["out_pre"][:, 0:2])
                nc.scalar.activation(
                    out=sqscr, in_=s["out_pre"], func=AF.Square, accum_out=sumsq
                )
                s["sumsq"] = sumsq
                s["mu2"] = mu2

            def stage_e3b(t):
                # var = sumsq/D - mu^2 ; rstd = exp(-0.5 ln(var+eps))
                s = st[t]
                varc = smalls.tile([P, 1], f32, tag="varc", name="varc")
                nc.vector.scalar_tensor_tensor(
                    out=varc, in0=s.pop("sumsq"), scalar=rD, in1=s.pop("mu2"),
                    op0=OP.mult, op1=OP.subtract,
                )
                lnv = smalls.tile([P, 1], f32, tag="lnv", name="lnv")
                nc.scalar.activation(
                    out=lnv, in_=varc, func=AF.Ln, bias=epsc, scale=1.0
                )
                rstd = smalls.tile([P, 1], f32, tag="rstd", name="rstd")
                nc.scalar.activation(out=rstd, in_=lnv, func=AF.Exp, scale=-0.5)
                s["rstd"] = rstd

            def stage_e4(t):
                # out = (out_pre - mu)*rstd
                s = st.pop(t)
                rstd = s["rstd"]
                nmr = smalls.tile([P, 1], f32, tag="nmr", name="nmr")
                nc.vector.tensor_scalar(
                    out=nmr, in0=s["mu"], scalar1=rstd, scalar2=-1.0,
                    op0=OP.mult, op1=OP.mult,
                )
                out_sb = work.tile([P, D], bf16, tag="out_sb", name="out_sb")
                nc.vector.tensor_scalar(
                    out=out_sb, in0=s["out_pre"], scalar1=rstd,
                    scalar2=nmr, op0=OP.mult, op1=OP.add,
                )
                touch_gp(out_sb[:, 0:2])
                nc.gpsimd.dma_start(out=o_t[t], in_=out_sb)

            stages = [
                (0, stage_in), (2, stage_a), (3, stage_b1), (4, stage_b2),
                (5, stage_c), (8, stage_d), (9, stage_d2e1), (10, stage_e2),
                (11, stage_e3), (12, stage_e3b), (13, stage_e4),
            ]
            total_lag = stages[-1][0]
            for i in range(ntiles + total_lag):
                for lag, fn in stages:
                    t = i - lag
                    if 0 <= t < ntiles:
                        fn(t)

    return nc


def _numpy_fallback(query, retrieved_memories, similarities, mask,
                    Wq, bq, Wk, bk, Wv, bv, Wo, bo, Wg, bg, ln_g, ln_b):
    x = query.astype(np.float64)
    m = retrieved_memories.astype(np.float64)
    q = x @ Wq + bq
    k = np.einsum("bkd,de->bke", m, Wk.astype(np.float64)) + bk
    v = np.einsum("bkd,de->bke", m, Wv.astype(np.float64)) + bv
    scores = np.einsum("bd,bkd->bk", q, k) * (D ** -0.5)
    scores = np.where(mask, scores, -np.inf)
    sm = scores - scores.max(-1, keepdims=True)
    w = np.exp(sm)
    w /= w.sum(-1, keepdims=True)
    w = np.where(mask, w, 0.0)
    mem = np.einsum("bk,bkd->bd", w, v) @ Wo + bo
    gate = 1 / (1 + np.exp(-(np.concatenate([x, mem], -1) @ Wg + bg)))
    conf = 1 / (1 + np.exp(-(similarities.max(-1, keepdims=True) - SIM_THRESH)))
    out = x + (gate * conf) * mem
    mu = out.mean(-1, keepdims=True)
    var = ((out - mu) ** 2).mean(-1, keepdims=True)
    out = (out - mu) / np.sqrt(var + LN_EPS) * ln_g + ln_b
    return out.astype(np.float32)


def kernel(**inputs):
    global LAST_RESULTS
    query = np.asarray(inputs["query"], dtype=np.float32)
    mem = np.asarray(inputs["retrieved_memories"], dtype=np.float32)
    sims = np.asarray(inputs["similarities"], dtype=np.float32)
    mask = np.asarray(inputs["mask"])
    Wq = np.asarray(inputs["Wq"], dtype=np.float64)
    Wk = np.asarray(inputs["Wk"], dtype=np.float64)
    Wv = np.asarray(inputs["Wv"], dtype=np.float64)
    Wo = np.asarray(inputs["Wo"], dtype=np.float64)
    Wg = np.asarray(inputs["Wg"], dtype=np.float64)

    # The device kernel folds all-zero biases / identity LN affine away.
    nontrivial = (
        any(np.any(np.asarray(inputs[n])) for n in ("bq", "bk", "bv", "bo", "bg"))
        or np.any(np.asarray(inputs["ln_b"]))
        or np.any(np.asarray(inputs["ln_g"]) != 1.0)
    )
    if nontrivial or query.shape != (B, D):
        return _numpy_fallback(
            query, mem, sims, mask, Wq=Wq, bq=np.asarray(inputs["bq"]),
            Wk=Wk, bk=np.asarray(inputs["bk"]), Wv=Wv, bv=np.asarray(inputs["bv"]),
            Wo=Wo, bo=np.asarray(inputs["bo"]), Wg=Wg, bg=np.asarray(inputs["bg"]),
            ln_g=np.asarray(inputs["ln_g"]), ln_b=np.asarray(inputs["ln_b"]),
        )

    import ml_dtypes
    bf = ml_dtypes.bfloat16
    wqk = np.ascontiguousarray(((Wq @ Wk.T) * (float(D) ** -0.5)).astype(bf))
    wvo64 = Wv @ Wo
    wvo = np.ascontiguousarray(wvo64.astype(bf))
    g1 = Wg[:D, 0]
    g2 = wvo64 @ Wg[D:, 0]
    ones_rD = np.full(D, 1.0 / D)
    gq = np.ascontiguousarray(np.stack([-g1, ones_rD], axis=1).astype(bf))
    gm = np.ascontiguousarray(
        np.stack([-g2, wvo64.sum(axis=1) / D], axis=1).astype(bf)
    )
    ident = np.eye(P, dtype=bf)

    if "nc" not in _CACHE:
        _CACHE["nc"] = _build()
    nc = _CACHE["nc"]

    qm = np.empty((B, (K + 1) * D), dtype=bf)
    qm[:, :D] = query
    qm[:, D:] = mem.reshape(B, K * D)
    maskf = np.ascontiguousarray(mask.astype(np.float32))
    conf = (1.0 / (1.0 + np.exp(-(sims.max(axis=-1) - SIM_THRESH)))).astype(
        np.float32
    ).reshape(B, 1)
    in_maps = []
    for c in range(N_CORES):
        sl = slice(c * ROWS, (c + 1) * ROWS)
        in_maps.append({
            "qm": qm[sl], "maskf": maskf[sl], "conf": conf[sl],
            "wqk": wqk, "wvo": wvo, "gq": gq, "gm": gm, "ident": ident,
        })

    from concourse.bass_utils import run_bass_kernel_spmd

    res = run_bass_kernel_spmd(nc, in_maps, list(range(N_CORES)), trace=TRACE)
    LAST_RESULTS = res
    return np.concatenate(
        [res.results[c]["o"].astype(np.float32) for c in range(N_CORES)], axis=0
    )
